# revision 39
# baseline (speedup 1.0000x reference)
"""Trainium2 Bass kernel for a 30-layer chain of UpDownDoubleResNet blocks.

Per layer (reference semantics, fp32):
    x1 = x1 + relu(x2) @ W1^T + b1      # [N,1], W1 [1,5]
    x2 = x2 + relu(x1) @ W2^T           # [N,5], W2 [5,1]

Strategy: pure data parallel over 8 NeuronCores.  Each core gets
524288 rows laid out as 128 partitions x 4096 free-dim elements, with
x2 pre-transposed on the host into 5 planar feature planes so that all
device access patterns are contiguous.  The whole per-core state lives
in SBUF (two chunks whose compute interleaves at half-layer granularity
to hide cross-engine latencies); each of the 30 layers runs:
  - 5 x scalar_tensor_tensor:  x1 = (relu_plane_j * w1_j) + x1     (DVE)
  - 1 x activation:            r  = relu(x1 + cumbias_k)           (ACT)
  - 5 x scalar_tensor_tensor:  plane_j = (r * w2_j) + plane_j      (DVE)
  - 5 x activation(Relu) producing relu planes for the next layer   (ACT,
    runs in the shadow of the DVE work)
The b1 bias is folded into the relu via host-side cumulative sums, so
the carried x1 state is bias-free until a single final correction.
Weights are per-partition scalar operands read from a small broadcast
table, so the compiled program is independent of weight values.

The walrus build in this container rejects instructions carrying more
than one sync-wait command, which shapes the tiny "warm-up", "tick
cover" and lane-rotation ops sprinkled through the program (each
absorbs exactly one semaphore wait so the real compute ops never need
two), plus a final pass that splits any remaining multi-wait
instruction onto injected Drains.
"""

import numpy as np

N_ROWS = 4_194_304
N_LAYERS = 30
N_CORES = 8
P = 128
ROWS_PER_CORE = N_ROWS // N_CORES  # 524288
F_TOT = ROWS_PER_CORE // P  # 4096
N_CHUNKS = 2
NW = 11  # weight-table cols per layer: 5 w1(|w1| in tt16), 5 w2, 1 cum-bias

# state dtype for on-device compute ("float32" or "float16")
STATE_DTYPE = "float32"
# compute mode: "stt" (scalar_tensor_tensor, best for fp32) or
# "tt16" (tensor_tensor combines at 2x + prescaled relu planes, for 16-bit)
MODE = "stt"

_BUILD_CACHE: dict = {}


def _mybir_dt(dtype_str):
    import concourse.mybir as mybir

    return {
        "float32": mybir.dt.float32,
        "float16": mybir.dt.float16,
        "bfloat16": mybir.dt.bfloat16,
    }[dtype_str]


def build_bass(
    dtype_str=STATE_DTYPE,
    f_tot=F_TOT,
    n_chunks=N_CHUNKS,
    n_layers=N_LAYERS,
    mode="stt",
    s1_signs=None,
):
    """Build the Bass program (shared by all 8 cores, SPMD).

    mode="stt": fused (relu_plane * w) + acc via scalar_tensor_tensor (1x DVE).
    mode="tt16": plain tensor_tensor combines (2x DVE for 16-bit dtypes) with
      relu planes pre-scaled by |w1| on ACT; w1 signs (s1_signs, a tuple of
      per-layer tuples derived from the runtime weights) select add/subtract.
    """
    key = (dtype_str, f_tot, n_chunks, n_layers, mode, s1_signs)
    if key in _BUILD_CACHE:
        return _BUILD_CACHE[key]
    assert mode in ("stt", "tt16")
    if mode == "tt16":
        assert s1_signs is not None and len(s1_signs) == n_layers

    import concourse.mybir as mybir
    from concourse.bass import Bass
    from concourse.tile import TileContext, add_dep_helper

    dt = _mybir_dt(dtype_str)
    f32 = mybir.dt.float32
    Relu = mybir.ActivationFunctionType.Relu
    mult = mybir.AluOpType.mult
    add = mybir.AluOpType.add
    sub = mybir.AluOpType.subtract
    amax = mybir.AluOpType.max

    F = f_tot // n_chunks
    nc = Bass("TRN2", debug=False, num_devices=N_CORES)

    # one extra all-zeros column at the end, used as activation bias AP so no
    # framework const tensors (with their own init deps) are ever referenced
    WTW = n_layers * NW + 1
    x1_d = nc.dram_tensor("x1", [P, f_tot], dt, kind="ExternalInput")
    x2_d = nc.dram_tensor("x2", [P, 5, f_tot], dt, kind="ExternalInput")
    wt_d = nc.dram_tensor("wt", [P, WTW], f32, kind="ExternalInput")
    x1o_d = nc.dram_tensor("x1o", [P, f_tot], dt, kind="ExternalOutput")
    x2o_d = nc.dram_tensor("x2o", [P, 5, f_tot], dt, kind="ExternalOutput")

    with TileContext(nc) as tc:
        with (
            tc.tile_pool(name="wpool", bufs=1) as wpool,
            tc.tile_pool(name="state", bufs=1) as state,
        ):
            wt = wpool.tile([P, WTW], f32)
            nc.sync.dma_start(out=wt[:], in_=wt_d.ap()[:, :])

            def wcol(idx):
                return wt[:, idx : idx + 1]

            zcol = wcol(WTW - 1)  # zeros, used as activation bias

            # ---- allocate + load all chunks up front (they stay resident;
            # the two chunks' compute is interleaved at layer granularity to
            # keep both engines busy across cross-engine dependencies) ----
            ch = []
            for c in range(n_chunks):
                sl = slice(c * F, (c + 1) * F)
                x1t = state.tile([P, F], dt, tag=f"x1t{c}", name=f"x1t{c}")
                x2t = state.tile([P, 5 * F], dt, tag=f"x2t{c}", name=f"x2t{c}")
                # half-plane loads, plane 0 first: the layer-0 relus can
                # start on each half-plane as soon as it arrives
                H = F // 2
                for h in range(2):
                    nc.sync.dma_start(
                        out=x2t[:, h * H : (h + 1) * H],
                        in_=x2_d.ap()[:, 0, c * F + h * H : c * F + (h + 1) * H])
                nc.sync.dma_start(out=x1t[:, :], in_=x1_d.ap()[:, sl])
                for j in range(1, 5):
                    for h in range(2):
                        nc.sync.dma_start(
                            out=x2t[:, j * F + h * H : j * F + (h + 1) * H],
                            in_=x2_d.ap()[:, j,
                                          c * F + h * H : c * F + (h + 1) * H])
                R = state.tile([P, 5 * F], dt, tag=f"R{c}", name=f"R{c}")
                r = state.tile([P, F], dt, tag=f"r{c}", name=f"r{c}")
                rp = (state.tile([P, F], dt, tag=f"rp{c}", name=f"rp{c}")
                      if mode == "tt16" else None)
                # tick-cover scratch: a fresh column per layer so cover ops
                # never acquire WAW deps of their own
                tkv = state.tile([P, n_layers + 2], dt, tag=f"tkv{c}",
                                 name=f"tkv{c}")
                tka = state.tile([P, 2 * n_layers + 2], dt, tag=f"tka{c}",
                                 name=f"tka{c}")
                warmv = state.tile([P, 4], f32, tag=f"warmv{c}", name=f"warmv{c}")
                warma = state.tile([P, 4], f32, tag=f"warma{c}", name=f"warma{c}")
                ch.append(dict(sl=sl, x1t=x1t, x2t=x2t, R=R, r=r, rp=rp,
                               tkv=tkv, tka=tka, warmv=warmv, warma=warma))

            def plane(t, j):
                return t[:, j * F : (j + 1) * F]

            # warm-up ops: absorb the DMA-completion semaphore waits so no
            # compute instruction ever needs more than one sync wait (the
            # TRN2 Activation / TensorScalarPtr instruction structs have a
            # single wait slot).  Chunk 0 warms up immediately; later chunks
            # warm up just before their first layer section so chunk-0
            # compute overlaps their DMA loads.
            def warm_chunk(c, t):
                nc.vector.tensor_copy(out=t["warmv"][:, 0:1], in_=t["x1t"][:, 0:1])
                if c == 0:
                    nc.vector.tensor_copy(out=t["warmv"][:, 2:3], in_=wt[:, 0:1])
                    nc.scalar.activation(t["warma"][:, 0:1], wt[:, 0:1], Relu,
                                         bias=zcol)
                # per-half-plane DVE warm copies (each absorbs that half's
                # DMA lane wait) + half-plane layer-0 relus (each carries its
                # own half's single DMA wait on ACT)
                H2 = F // 2
                for j in range(5):
                    for h in range(2):
                        lo = j * F + h * H2
                        nc.vector.tensor_copy(
                            out=t["warmv"][:, 1:2],
                            in_=t["x2t"][:, lo : lo + 1])
                        nc.scalar.activation(
                            t["R"][:, lo : lo + H2],
                            t["x2t"][:, lo : lo + H2], Relu,
                            bias=zcol, scale=wcol(j) if mode == "tt16" else 1.0)

            warm_chunk(0, ch[0])

            for k in range(n_layers):
                base = k * NW
                # phase 1 per chunk: dot chain on DVE; ACT own-cover + r.
                # phase 2 per chunk: plane updates on DVE; ACT relu planes.
                # The two chunks' phases interleave so every cross-engine
                # dependency (r, relu planes) is hidden behind the other
                # chunk's DVE work.
                for c, t in enumerate(ch):
                    if k == 0 and c > 0:
                        warm_chunk(c, t)
                    x1t, x2t, R, r = t["x1t"], t["x2t"], t["R"], t["r"]
                    # ACT own-tick cover, then r = relu(x1 + cumbias_k) on ACT
                    t["cov_a"] = nc.scalar.activation(
                        t["tka"][:, 2 * k : 2 * k + 1],
                        R[:, 5 * F - 1 : 5 * F], Relu, bias=zcol)
                    # dot chain, accumulated in place into x1.  Plane 4 is
                    # consumed FIRST: its relu was emitted last in the
                    # previous layer, so the first dot op's single Activation
                    # wait covers all five relu-plane ticks (no DVE tick-cover
                    # op needed).
                    for j in (4, 3, 2, 1, 0):
                        if mode == "stt":
                            nc.vector.scalar_tensor_tensor(
                                out=x1t[:, :],
                                in0=plane(R, j),
                                scalar=wcol(base + j),
                                in1=x1t[:, :],
                                op0=mult,
                                op1=add,
                            )
                        else:
                            nc.vector.tensor_tensor(
                                out=x1t[:, :],
                                in0=x1t[:, :],
                                in1=plane(R, j),
                                op=add if s1_signs[k][j] else sub,
                            )
                    ra = nc.scalar.activation(r[:, :], x1t[:, :], Relu,
                                              bias=wcol(base + 10))
                    add_dep_helper(ra.ins, t["cov_a"].ins, sync=False,
                                   reason="cover before r")
                    if k + 1 == n_layers:
                        # final x1 = x1_state + cumbias written into the (now
                        # dead) R scratch so the store can't race the r relu;
                        # dispatches while both chunks still compute
                        nc.vector.tensor_scalar(
                            out=R[:, 0:F],
                            in0=x1t[:, :],
                            scalar1=wcol(base + 10),
                            scalar2=None,
                            op0=add,
                        )
                        nc.sync.dma_start(out=x1o_d.ap()[:, t["sl"]],
                                          in_=R[:, 0:F])
                for c, t in enumerate(ch):
                    x1t, x2t, R, r, rp = (t["x1t"], t["x2t"], t["R"], t["r"],
                                          t["rp"])
                    # plane updates
                    for j in range(5):
                        if mode == "stt":
                            nc.vector.scalar_tensor_tensor(
                                out=plane(x2t, j),
                                in0=r[:, :],
                                scalar=wcol(base + 5 + j),
                                in1=plane(x2t, j),
                                op0=mult,
                                op1=add,
                            )
                        else:
                            nc.vector.tensor_scalar(
                                out=rp[:, :],
                                in0=r[:, :],
                                scalar1=wcol(base + 5 + j),
                                scalar2=None,
                                op0=mult,
                            )
                            nc.vector.tensor_tensor(
                                out=plane(x2t, j),
                                in0=plane(x2t, j),
                                in1=rp[:, :],
                                op=add,
                            )
                        if k + 1 == n_layers:
                            # last layer: store this plane the moment its
                            # final value lands (multi-wait DMAs are
                            # legalised by _split_multiwait)
                            nc.sync.dma_start(
                                out=x2o_d.ap()[:, j, t["sl"]],
                                in_=plane(x2t, j),
                            )
                    if k + 1 < n_layers:
                        # second ACT cover (after the plane updates): absorbs
                        # the DVE tick of the last plane update, so the relus
                        # themselves need no waits at all
                        cov_a2 = nc.scalar.activation(
                            t["tka"][:, 2 * k + 1 : 2 * k + 2],
                            x2t[:, 5 * F - 1 : 5 * F], Relu, bias=zcol)
                        add_dep_helper(cov_a2.ins, t["cov_a"].ins, sync=False,
                                       reason="cover order")
                        # relu planes for the next layer
                        for j in range(5):
                            rl = nc.scalar.activation(
                                plane(R, j), plane(x2t, j), Relu,
                                bias=zcol,
                                scale=wcol((k + 1) * NW + j)
                                if mode == "tt16" else 1.0,
                            )
                            add_dep_helper(rl.ins, cov_a2.ins, sync=False,
                                           reason="cover before relu")

    _split_multiwait(nc)
    _BUILD_CACHE[key] = nc
    return nc


def _split_multiwait(nc, max_waits=1):
    """The walrus build in this container rejects instructions whose struct
    carries more than one sync-wait command.  Hoist extra waits onto injected
    same-engine Drain instructions placed immediately before the offender."""
    import concourse.mybir as mybir

    n_split = 0
    for f in nc.m.functions:
        for bb in f.blocks:
            new_insts = []
            changed = False
            for ins in bb.instructions:
                si = ins.sync_info
                waits = list(si.on_wait) if si is not None else []
                if len(waits) > max_waits:
                    extra, keep = waits[:-max_waits], waits[-max_waits:]
                    for w in extra:
                        d = mybir.InstDrain(
                            name=f"{ins.name}_wsplit{n_split}",
                            ins=[],
                            outs=[],
                            sync_info=mybir.SyncInfo(on_wait=[w], on_update=[]),
                        )
                        d.engine = ins.engine
                        new_insts.append(d)
                        n_split += 1
                    ins.sync_info = mybir.SyncInfo(
                        on_wait=keep, on_update=list(si.on_update)
                    )
                    changed = True
                new_insts.append(ins)
            if changed:
                bb.instructions = new_insts
    return n_split


def _host_prep(x1, x2, W1, b1, W2, dtype_str, f_tot=F_TOT, n_cores=N_CORES,
               mode="stt"):
    """Shard + lay out inputs per core; build the weight table."""
    np_dt = np.dtype(dtype_str)
    rows_per_core = x1.shape[0] // n_cores
    n_layers = W1.shape[0]

    # weight table, broadcast to all partitions: [P, n_layers*NW + 1] fp32
    # (the final column is zeros, used as the activation bias AP).
    # In tt16 mode the w1 columns hold |w1| (the sign is baked into the
    # add/subtract choice of the combine instruction).
    wrow = np.zeros(n_layers * NW + 1, dtype=np.float32)
    cumb = np.cumsum(b1[:, 0].astype(np.float64)).astype(np.float32)
    for k in range(n_layers):
        w1k = W1[k, 0, :]
        wrow[k * NW : k * NW + 5] = np.abs(w1k) if mode == "tt16" else w1k
        wrow[k * NW + 5 : k * NW + 10] = W2[k, :, 0]
        wrow[k * NW + 10] = cumb[k]
    wtab = np.ascontiguousarray(np.broadcast_to(wrow, (P, wrow.shape[0])))

    in_maps = []
    for c in range(n_cores):
        lo = c * rows_per_core
        hi = lo + rows_per_core
        x1c = np.ascontiguousarray(
            x1[lo:hi, 0].reshape(P, f_tot).astype(np_dt, copy=False)
        )
        # [rows,5] -> [P, F_TOT, 5] -> [P, 5, F_TOT]
        x2c = np.ascontiguousarray(
            x2[lo:hi].reshape(P, f_tot, 5).transpose(0, 2, 1).astype(np_dt, copy=False)
        )
        in_maps.append({"x1": x1c, "x2": x2c, "wt": wtab})
    return in_maps


def _host_unprep(results, dtype_out=np.float32, f_tot=F_TOT, n_cores=N_CORES):
    rows_per_core = P * f_tot
    x1 = np.empty((n_cores * rows_per_core, 1), dtype=dtype_out)
    x2 = np.empty((n_cores * rows_per_core, 5), dtype=dtype_out)
    for c, res in enumerate(results):
        lo = c * rows_per_core
        hi = lo + rows_per_core
        x1[lo:hi, 0] = res["x1o"].astype(dtype_out).reshape(-1)
        x2[lo:hi] = (
            res["x2o"].astype(dtype_out).transpose(0, 2, 1).reshape(rows_per_core, 5)
        )
    return x1, x2


def _run(inputs, dtype_str=STATE_DTYPE, mode=None, trace=False):
    from concourse import bass_utils

    if mode is None:
        mode = MODE
    s1_signs = None
    if mode == "tt16":
        s1_signs = tuple(
            tuple(bool(v) for v in (inputs["W1"][k, 0, :] >= 0))
            for k in range(inputs["W1"].shape[0])
        )
    nc = build_bass(dtype_str, mode=mode, s1_signs=s1_signs)
    in_maps = _host_prep(
        inputs["x1"], inputs["x2"], inputs["W1"], inputs["b1"], inputs["W2"],
        dtype_str, mode=mode,
    )
    res = bass_utils.run_bass_kernel_spmd(
        nc, in_maps, core_ids=list(range(N_CORES)), trace=trace
    )
    x1, x2 = _host_unprep(res.results)
    return (x1, x2), res


def kernel(x1, x2, W1, b1, W2):
    inputs = {
        k: np.asarray(v)
        for k, v in {"x1": x1, "x2": x2, "W1": W1, "b1": b1, "W2": W2}.items()
    }
    (x1o, x2o), _ = _run(inputs)
    return x1o, x2o


# revision 40
# speedup vs baseline: 1.0009x; 1.0009x over previous
"""Trainium2 Bass kernel for a 30-layer chain of UpDownDoubleResNet blocks.

Per layer (reference semantics, fp32):
    x1 = x1 + relu(x2) @ W1^T + b1      # [N,1], W1 [1,5]
    x2 = x2 + relu(x1) @ W2^T           # [N,5], W2 [5,1]

Strategy: pure data parallel over 8 NeuronCores.  Each core gets
524288 rows laid out as 128 partitions x 4096 free-dim elements, with
x2 pre-transposed on the host into 5 planar feature planes so that all
device access patterns are contiguous.  The whole per-core state lives
in SBUF (two chunks whose compute interleaves at half-layer granularity
to hide cross-engine latencies); each of the 30 layers runs:
  - 5 x scalar_tensor_tensor:  x1 = (relu_plane_j * w1_j) + x1     (DVE)
  - 1 x activation:            r  = relu(x1 + cumbias_k)           (ACT)
  - 5 x scalar_tensor_tensor:  plane_j = (r * w2_j) + plane_j      (DVE)
  - 5 x activation(Relu) producing relu planes for the next layer   (ACT,
    runs in the shadow of the DVE work)
The b1 bias is folded into the relu via host-side cumulative sums, so
the carried x1 state is bias-free until a single final correction.
Weights are per-partition scalar operands read from a small broadcast
table, so the compiled program is independent of weight values.

The walrus build in this container rejects instructions carrying more
than one sync-wait command, which shapes the tiny "warm-up", "tick
cover" and lane-rotation ops sprinkled through the program (each
absorbs exactly one semaphore wait so the real compute ops never need
two), plus a final pass that splits any remaining multi-wait
instruction onto injected Drains.
"""

import numpy as np

N_ROWS = 4_194_304
N_LAYERS = 30
N_CORES = 8
P = 128
ROWS_PER_CORE = N_ROWS // N_CORES  # 524288
F_TOT = ROWS_PER_CORE // P  # 4096
N_CHUNKS = 2
NW = 11  # weight-table cols per layer: 5 w1(|w1| in tt16), 5 w2, 1 cum-bias

# state dtype for on-device compute ("float32" or "float16")
STATE_DTYPE = "float32"
# compute mode: "stt" (scalar_tensor_tensor, best for fp32) or
# "tt16" (tensor_tensor combines at 2x + prescaled relu planes, for 16-bit)
MODE = "stt"

_BUILD_CACHE: dict = {}


def _mybir_dt(dtype_str):
    import concourse.mybir as mybir

    return {
        "float32": mybir.dt.float32,
        "float16": mybir.dt.float16,
        "bfloat16": mybir.dt.bfloat16,
    }[dtype_str]


def build_bass(
    dtype_str=STATE_DTYPE,
    f_tot=F_TOT,
    n_chunks=N_CHUNKS,
    n_layers=N_LAYERS,
    mode="stt",
    s1_signs=None,
):
    """Build the Bass program (shared by all 8 cores, SPMD).

    mode="stt": fused (relu_plane * w) + acc via scalar_tensor_tensor (1x DVE).
    mode="tt16": plain tensor_tensor combines (2x DVE for 16-bit dtypes) with
      relu planes pre-scaled by |w1| on ACT; w1 signs (s1_signs, a tuple of
      per-layer tuples derived from the runtime weights) select add/subtract.
    """
    key = (dtype_str, f_tot, n_chunks, n_layers, mode, s1_signs)
    if key in _BUILD_CACHE:
        return _BUILD_CACHE[key]
    assert mode in ("stt", "tt16")
    if mode == "tt16":
        assert s1_signs is not None and len(s1_signs) == n_layers

    import concourse.mybir as mybir
    from concourse.bass import Bass
    from concourse.tile import TileContext, add_dep_helper

    dt = _mybir_dt(dtype_str)
    f32 = mybir.dt.float32
    Relu = mybir.ActivationFunctionType.Relu
    mult = mybir.AluOpType.mult
    add = mybir.AluOpType.add
    sub = mybir.AluOpType.subtract
    amax = mybir.AluOpType.max

    F = f_tot // n_chunks
    nc = Bass("TRN2", debug=False, num_devices=N_CORES)

    # one extra all-zeros column at the end, used as activation bias AP so no
    # framework const tensors (with their own init deps) are ever referenced
    WTW = n_layers * NW + 1
    x1_d = nc.dram_tensor("x1", [P, f_tot], dt, kind="ExternalInput")
    x2_d = nc.dram_tensor("x2", [P, 5, f_tot], dt, kind="ExternalInput")
    wt_d = nc.dram_tensor("wt", [P, WTW], f32, kind="ExternalInput")
    x1o_d = nc.dram_tensor("x1o", [P, f_tot], dt, kind="ExternalOutput")
    x2o_d = nc.dram_tensor("x2o", [P, 5, f_tot], dt, kind="ExternalOutput")

    with TileContext(nc) as tc:
        with (
            tc.tile_pool(name="wpool", bufs=1) as wpool,
            tc.tile_pool(name="state", bufs=1) as state,
        ):
            wt = wpool.tile([P, WTW], f32)
            nc.sync.dma_start(out=wt[:], in_=wt_d.ap()[:, :])

            def wcol(idx):
                return wt[:, idx : idx + 1]

            zcol = wcol(WTW - 1)  # zeros, used as activation bias

            # ---- allocate + load all chunks up front (they stay resident;
            # the two chunks' compute is interleaved at layer granularity to
            # keep both engines busy across cross-engine dependencies) ----
            ch = []
            for c in range(n_chunks):
                sl = slice(c * F, (c + 1) * F)
                x1t = state.tile([P, F], dt, tag=f"x1t{c}", name=f"x1t{c}")
                x2t = state.tile([P, 5 * F], dt, tag=f"x2t{c}", name=f"x2t{c}")
                # per-plane loads, plane 0 first: the layer-0 relus can
                # start as soon as their plane arrives
                nc.sync.dma_start(
                    out=x2t[:, 0:F], in_=x2_d.ap()[:, 0, sl])
                nc.sync.dma_start(out=x1t[:, :], in_=x1_d.ap()[:, sl])
                for j in range(1, 5):
                    nc.sync.dma_start(
                        out=x2t[:, j * F : (j + 1) * F],
                        in_=x2_d.ap()[:, j, sl],
                    )
                R = state.tile([P, 5 * F], dt, tag=f"R{c}", name=f"R{c}")
                r = state.tile([P, F], dt, tag=f"r{c}", name=f"r{c}")
                rp = (state.tile([P, F], dt, tag=f"rp{c}", name=f"rp{c}")
                      if mode == "tt16" else None)
                # tick-cover scratch: a fresh column per layer so cover ops
                # never acquire WAW deps of their own
                tkv = state.tile([P, n_layers + 2], dt, tag=f"tkv{c}",
                                 name=f"tkv{c}")
                tka = state.tile([P, 2 * n_layers + 2], dt, tag=f"tka{c}",
                                 name=f"tka{c}")
                warmv = state.tile([P, 4], f32, tag=f"warmv{c}", name=f"warmv{c}")
                warma = state.tile([P, 4], f32, tag=f"warma{c}", name=f"warma{c}")
                ch.append(dict(sl=sl, x1t=x1t, x2t=x2t, R=R, r=r, rp=rp,
                               tkv=tkv, tka=tka, warmv=warmv, warma=warma))

            def plane(t, j):
                return t[:, j * F : (j + 1) * F]

            # warm-up ops: absorb the DMA-completion semaphore waits so no
            # compute instruction ever needs more than one sync wait (the
            # TRN2 Activation / TensorScalarPtr instruction structs have a
            # single wait slot).  Chunk 0 warms up immediately; later chunks
            # warm up just before their first layer section so chunk-0
            # compute overlaps their DMA loads.
            def warm_chunk(c, t):
                nc.vector.tensor_copy(out=t["warmv"][:, 0:1], in_=t["x1t"][:, 0:1])
                if c == 0:
                    nc.vector.tensor_copy(out=t["warmv"][:, 2:3], in_=wt[:, 0:1])
                    nc.scalar.activation(t["warma"][:, 0:1], wt[:, 0:1], Relu,
                                         bias=zcol)
                # per-plane DVE warm copies (each absorbs that plane's DMA
                # lane wait) + per-plane layer-0 relus (each carries its own
                # plane's single DMA wait on ACT)
                for j in range(5):
                    nc.vector.tensor_copy(
                        out=t["warmv"][:, 1:2],
                        in_=t["x2t"][:, j * F : j * F + 1])
                    nc.scalar.activation(
                        plane(t["R"], j), plane(t["x2t"], j), Relu,
                        bias=zcol, scale=wcol(j) if mode == "tt16" else 1.0)

            warm_chunk(0, ch[0])

            for k in range(n_layers):
                base = k * NW
                # phase 1 per chunk: dot chain on DVE; ACT own-cover + r.
                # phase 2 per chunk: plane updates on DVE; ACT relu planes.
                # The two chunks' phases interleave so every cross-engine
                # dependency (r, relu planes) is hidden behind the other
                # chunk's DVE work.
                for c, t in enumerate(ch):
                    if k == 0 and c > 0:
                        warm_chunk(c, t)
                    x1t, x2t, R, r = t["x1t"], t["x2t"], t["R"], t["r"]
                    # ACT own-tick cover, then r = relu(x1 + cumbias_k) on ACT
                    t["cov_a"] = nc.scalar.activation(
                        t["tka"][:, 2 * k : 2 * k + 1],
                        R[:, 5 * F - 1 : 5 * F], Relu, bias=zcol)
                    # dot chain, accumulated in place into x1.  Plane 4 is
                    # consumed FIRST: its relu was emitted last in the
                    # previous layer, so the first dot op's single Activation
                    # wait covers all five relu-plane ticks (no DVE tick-cover
                    # op needed).
                    for j in (4, 3, 2, 1, 0):
                        if mode == "stt":
                            nc.vector.scalar_tensor_tensor(
                                out=x1t[:, :],
                                in0=plane(R, j),
                                scalar=wcol(base + j),
                                in1=x1t[:, :],
                                op0=mult,
                                op1=add,
                            )
                        else:
                            nc.vector.tensor_tensor(
                                out=x1t[:, :],
                                in0=x1t[:, :],
                                in1=plane(R, j),
                                op=add if s1_signs[k][j] else sub,
                            )
                    ra = nc.scalar.activation(r[:, :], x1t[:, :], Relu,
                                              bias=wcol(base + 10))
                    add_dep_helper(ra.ins, t["cov_a"].ins, sync=False,
                                   reason="cover before r")
                    if k + 1 == n_layers:
                        # final x1 = x1_state + cumbias written into the (now
                        # dead) R scratch so the store can't race the r relu;
                        # dispatches while both chunks still compute
                        nc.vector.tensor_scalar(
                            out=R[:, 0:F],
                            in0=x1t[:, :],
                            scalar1=wcol(base + 10),
                            scalar2=None,
                            op0=add,
                        )
                        nc.sync.dma_start(out=x1o_d.ap()[:, t["sl"]],
                                          in_=R[:, 0:F])
                for c, t in enumerate(ch):
                    x1t, x2t, R, r, rp = (t["x1t"], t["x2t"], t["R"], t["r"],
                                          t["rp"])
                    # plane updates
                    for j in range(5):
                        if mode == "stt":
                            nc.vector.scalar_tensor_tensor(
                                out=plane(x2t, j),
                                in0=r[:, :],
                                scalar=wcol(base + 5 + j),
                                in1=plane(x2t, j),
                                op0=mult,
                                op1=add,
                            )
                        else:
                            nc.vector.tensor_scalar(
                                out=rp[:, :],
                                in0=r[:, :],
                                scalar1=wcol(base + 5 + j),
                                scalar2=None,
                                op0=mult,
                            )
                            nc.vector.tensor_tensor(
                                out=plane(x2t, j),
                                in0=plane(x2t, j),
                                in1=rp[:, :],
                                op=add,
                            )
                        if k + 1 == n_layers:
                            # last layer: store this plane the moment its
                            # final value lands (multi-wait DMAs are
                            # legalised by _split_multiwait)
                            nc.sync.dma_start(
                                out=x2o_d.ap()[:, j, t["sl"]],
                                in_=plane(x2t, j),
                            )
                    if k + 1 < n_layers:
                        # second ACT cover (after the plane updates): absorbs
                        # the DVE tick of the last plane update, so the relus
                        # themselves need no waits at all
                        cov_a2 = nc.scalar.activation(
                            t["tka"][:, 2 * k + 1 : 2 * k + 2],
                            x2t[:, 5 * F - 1 : 5 * F], Relu, bias=zcol)
                        add_dep_helper(cov_a2.ins, t["cov_a"].ins, sync=False,
                                       reason="cover order")
                        # relu planes for the next layer
                        for j in range(5):
                            rl = nc.scalar.activation(
                                plane(R, j), plane(x2t, j), Relu,
                                bias=zcol,
                                scale=wcol((k + 1) * NW + j)
                                if mode == "tt16" else 1.0,
                            )
                            add_dep_helper(rl.ins, cov_a2.ins, sync=False,
                                           reason="cover before relu")

    _split_multiwait(nc)
    _BUILD_CACHE[key] = nc
    return nc


def _split_multiwait(nc, max_waits=1):
    """The walrus build in this container rejects instructions whose struct
    carries more than one sync-wait command.  Hoist extra waits onto injected
    same-engine Drain instructions placed immediately before the offender."""
    import concourse.mybir as mybir

    n_split = 0
    for f in nc.m.functions:
        for bb in f.blocks:
            new_insts = []
            changed = False
            for ins in bb.instructions:
                si = ins.sync_info
                waits = list(si.on_wait) if si is not None else []
                if len(waits) > max_waits:
                    extra, keep = waits[:-max_waits], waits[-max_waits:]
                    for w in extra:
                        d = mybir.InstDrain(
                            name=f"{ins.name}_wsplit{n_split}",
                            ins=[],
                            outs=[],
                            sync_info=mybir.SyncInfo(on_wait=[w], on_update=[]),
                        )
                        d.engine = ins.engine
                        new_insts.append(d)
                        n_split += 1
                    ins.sync_info = mybir.SyncInfo(
                        on_wait=keep, on_update=list(si.on_update)
                    )
                    changed = True
                new_insts.append(ins)
            if changed:
                bb.instructions = new_insts
    return n_split


def _host_prep(x1, x2, W1, b1, W2, dtype_str, f_tot=F_TOT, n_cores=N_CORES,
               mode="stt"):
    """Shard + lay out inputs per core; build the weight table."""
    np_dt = np.dtype(dtype_str)
    rows_per_core = x1.shape[0] // n_cores
    n_layers = W1.shape[0]

    # weight table, broadcast to all partitions: [P, n_layers*NW + 1] fp32
    # (the final column is zeros, used as the activation bias AP).
    # In tt16 mode the w1 columns hold |w1| (the sign is baked into the
    # add/subtract choice of the combine instruction).
    wrow = np.zeros(n_layers * NW + 1, dtype=np.float32)
    cumb = np.cumsum(b1[:, 0].astype(np.float64)).astype(np.float32)
    for k in range(n_layers):
        w1k = W1[k, 0, :]
        wrow[k * NW : k * NW + 5] = np.abs(w1k) if mode == "tt16" else w1k
        wrow[k * NW + 5 : k * NW + 10] = W2[k, :, 0]
        wrow[k * NW + 10] = cumb[k]
    wtab = np.ascontiguousarray(np.broadcast_to(wrow, (P, wrow.shape[0])))

    in_maps = []
    for c in range(n_cores):
        lo = c * rows_per_core
        hi = lo + rows_per_core
        x1c = np.ascontiguousarray(
            x1[lo:hi, 0].reshape(P, f_tot).astype(np_dt, copy=False)
        )
        # [rows,5] -> [P, F_TOT, 5] -> [P, 5, F_TOT]
        x2c = np.ascontiguousarray(
            x2[lo:hi].reshape(P, f_tot, 5).transpose(0, 2, 1).astype(np_dt, copy=False)
        )
        in_maps.append({"x1": x1c, "x2": x2c, "wt": wtab})
    return in_maps


def _host_unprep(results, dtype_out=np.float32, f_tot=F_TOT, n_cores=N_CORES):
    rows_per_core = P * f_tot
    x1 = np.empty((n_cores * rows_per_core, 1), dtype=dtype_out)
    x2 = np.empty((n_cores * rows_per_core, 5), dtype=dtype_out)
    for c, res in enumerate(results):
        lo = c * rows_per_core
        hi = lo + rows_per_core
        x1[lo:hi, 0] = res["x1o"].astype(dtype_out).reshape(-1)
        x2[lo:hi] = (
            res["x2o"].astype(dtype_out).transpose(0, 2, 1).reshape(rows_per_core, 5)
        )
    return x1, x2


def _run(inputs, dtype_str=STATE_DTYPE, mode=None, trace=False):
    from concourse import bass_utils

    if mode is None:
        mode = MODE
    s1_signs = None
    if mode == "tt16":
        s1_signs = tuple(
            tuple(bool(v) for v in (inputs["W1"][k, 0, :] >= 0))
            for k in range(inputs["W1"].shape[0])
        )
    nc = build_bass(dtype_str, mode=mode, s1_signs=s1_signs)
    in_maps = _host_prep(
        inputs["x1"], inputs["x2"], inputs["W1"], inputs["b1"], inputs["W2"],
        dtype_str, mode=mode,
    )
    res = bass_utils.run_bass_kernel_spmd(
        nc, in_maps, core_ids=list(range(N_CORES)), trace=trace
    )
    x1, x2 = _host_unprep(res.results)
    return (x1, x2), res


def kernel(x1, x2, W1, b1, W2):
    inputs = {
        k: np.asarray(v)
        for k, v in {"x1": x1, "x2": x2, "W1": W1, "b1": b1, "W2": W2}.items()
    }
    (x1o, x2o), _ = _run(inputs)
    return x1o, x2o
